# revision 1
# baseline (speedup 1.0000x reference)
"""Causal attention with memory + post-softmax expire gating, on 8 trn2 cores.

Sharding: batch (2) x head-groups (4 heads each) -> 8 cores. Each core
computes q/k/v projections for its 4 heads (column-parallel), local
attention, and a partial output projection (row-parallel over heads).
Host sums the 4 partial products per batch and adds the bias.

Device layout notes:
  - everything is computed transposed ("d-major"): ctx^T [dim, j] feeds
    the projections, S^T [j, i] makes the softmax denominator a ones-
    matmul and PV a plain matmul (no transposes anywhere).
  - softmax skips max-subtraction (|S*scale| <= ~5 for this data), so
    P = exp(scale*S) directly from PSUM on the ACT engine.
  - causal mask applied as a bf16 multiply on the 4 partial-block
    patterns; expire folded into v'.
"""

import numpy as np
import ml_dtypes
from contextlib import ExitStack

import concourse.bass as bass
import concourse.mybir as mybir
import concourse.tile as tile
from concourse import bacc
from concourse.bass_utils import run_bass_kernel_spmd

F32 = mybir.dt.float32
F32R = mybir.dt.float32r
BF16 = mybir.dt.bfloat16
AF = mybir.ActivationFunctionType
MULT = mybir.AluOpType.mult

HEADS = 16
B, N, MEM, DIM = 2, 2048, 2048, 1024
J = MEM + N                      # 4096
DH = 64                          # head dim
HPC = 4                          # heads per core
DHC = HPC * DH                   # 256 dims per core
SCALE = DH ** -0.5
NCORES = 8

NJB = J // 128                   # 32 j-blocks
NIT = N // 512                   # 4 i-blocks
NDB = DIM // 128                 # 8 D-blocks

ABLATE = set()                   # test-only: {"nol"} drops denominator matmuls
BF16S = True                     # store qT/kT in bf16 (separate-LDW matmul path)
REPS = 1                         # test-only: on-device repeat count for timing


def build_program_v(ablate=frozenset(), reps=1, bf16s=False):
    global ABLATE, REPS, BF16S
    old = (ABLATE, REPS, BF16S)
    ABLATE, REPS, BF16S = set(ablate), reps, bf16s
    try:
        return build_program()
    finally:
        ABLATE, REPS, BF16S = old


def _njb(it):
    # j valid iff j <= i + MEM; for i-block [512it, 512it+512):
    # j-blocks 0 .. 4it+19 (inclusive) are at least partially valid.
    return 4 * it + 20


def _off(it, jb):
    # mask offset; partial block iff 0 <= off < 512 (off multiple of 128)
    return 128 * jb - MEM - 512 * it


def build_program():
    nc = bacc.Bacc("TRN2", target_bir_lowering=False, debug=False,
                   num_devices=NCORES)
    ctxT_d = nc.dram_tensor("ctxT", [DIM, J], BF16, kind="ExternalInput").ap()
    wq_d = nc.dram_tensor("wq", [DIM, DHC], BF16, kind="ExternalInput").ap()
    wk_d = nc.dram_tensor("wk", [DIM, DHC], BF16, kind="ExternalInput").ap()
    wv_d = nc.dram_tensor("wv", [DIM, DHC], BF16, kind="ExternalInput").ap()
    wo_d = nc.dram_tensor("wo", [DHC, DIM], BF16, kind="ExternalInput").ap()
    exp_d = nc.dram_tensor("expire", [NJB, 128], F32, kind="ExternalInput").ap()
    msk_d = nc.dram_tensor("masks", [4, 128, 512], BF16, kind="ExternalInput").ap()
    out_d = nc.dram_tensor("out", [N, DIM], F32, kind="ExternalOutput").ap()

    with tile.TileContext(nc) as tc, ExitStack() as ctx:
        sb = ctx.enter_context(tc.tile_pool(name="sb", bufs=1))
        pb = ctx.enter_context(tc.tile_pool(name="pb", bufs=6))
        ob = ctx.enter_context(tc.tile_pool(name="ob", bufs=2))
        pp = ctx.enter_context(tc.tile_pool(name="pp", bufs=2, space="PSUM"))
        dp = ctx.enter_context(tc.tile_pool(name="dp", bufs=2, space="DRAM"))

        # ---- constants / inputs ----
        expire = sb.tile([128, NJB], F32)
        nc.sync.dma_start(out=expire, in_=exp_d.rearrange("j p -> p j"))
        masks = sb.tile([128, 4, 512], BF16)
        nc.sync.dma_start(out=masks, in_=msk_d.rearrange("o p i -> p o i"))
        ones = sb.tile([128, 1], BF16)
        nc.vector.memset(ones, 1.0)

        wq = sb.tile([128, NDB, DHC], BF16)
        wk = sb.tile([128, NDB, DHC], BF16)
        wv = sb.tile([128, NDB, DHC], BF16)
        nc.sync.dma_start(out=wq, in_=wq_d.rearrange("(db p) m -> p db m", p=128))
        nc.sync.dma_start(out=wk, in_=wk_d.rearrange("(db p) m -> p db m", p=128))
        nc.sync.dma_start(out=wv, in_=wv_d.rearrange("(db p) m -> p db m", p=128))
        wo = sb.tile([128, 2, DIM], BF16)
        nc.sync.dma_start(out=wo, in_=wo_d.rearrange("(pr p) m -> p pr m", p=128))

        rep_cm = tc.For_i(0, REPS, 1) if REPS > 1 else None
        if rep_cm is not None:
            rep_cm.__enter__()

        cx = sb.tile([128, NDB, J], BF16)
        for db in range(NDB):
            nc.sync.dma_start(out=cx[:, db, :], in_=ctxT_d[128 * db:128 * db + 128, :])

        # ---- projections ----
        # qT/kT pair-tiles: rows 0:64 = even head, 64:128 = odd head of pair
        qk_dt = BF16 if BF16S else F32R
        qT = [sb.tile([128, N], qk_dt, name=f"qT{p}", tag=f"qT{p}") for p in range(2)]
        kT = [sb.tile([128, J], qk_dt, name=f"kT{p}", tag=f"kT{p}") for p in range(2)]
        vp = sb.tile([128, NJB, DHC], BF16)      # v' = v * expire, [j-in-blk, jb, d]

        for pr in range(2):
            for it in range(NIT):
                ps = pp.tile([128, 512], F32, name="ps_q", tag="A", bufs=2)
                for db in range(NDB):
                    nc.tensor.matmul(
                        ps, lhsT=wq[:, db, 128 * pr:128 * pr + 128],
                        rhs=cx[:, db, MEM + 512 * it:MEM + 512 * it + 512],
                        start=(db == 0), stop=(db == NDB - 1))
                nc.vector.tensor_copy(out=qT[pr][:, 512 * it:512 * it + 512], in_=ps)
            for jt in range(J // 512):
                ps = pp.tile([128, 512], F32, name="ps_k", tag="A", bufs=2)
                for db in range(NDB):
                    nc.tensor.matmul(
                        ps, lhsT=wk[:, db, 128 * pr:128 * pr + 128],
                        rhs=cx[:, db, 512 * jt:512 * jt + 512],
                        start=(db == 0), stop=(db == NDB - 1))
                nc.vector.tensor_copy(out=kT[pr][:, 512 * jt:512 * jt + 512], in_=ps)
        for jb in range(NJB):
            ps = pp.tile([128, DHC], F32, name="ps_v", tag="A", bufs=2)
            for db in range(NDB):
                nc.tensor.matmul(
                    ps, lhsT=cx[:, db, 128 * jb:128 * jb + 128],
                    rhs=wv[:, db, :],
                    start=(db == 0), stop=(db == NDB - 1))
            nc.vector.tensor_scalar(out=vp[:, jb, :], in0=ps,
                                    scalar1=expire[:, jb:jb + 1], scalar2=None,
                                    op0=MULT)

        # ---- attention ----
        # attn_out^T, scaled by 1/l: pair tiles [128, N]
        ao = [sb.tile([128, N], BF16, name=f"ao{p}", tag=f"ao{p}") for p in range(2)]

        for it in range(NIT):
            njb = _njb(it)
            i0 = 512 * it
            isl = slice(i0, i0 + 512)
            lp = pp.tile([128, 512], F32, name="lp", tag="A", bufs=2)
            pv = [pp.tile([128, 512], F32, name=f"pv{p}", tag=f"pv{p}", bufs=1)
                  for p in range(2)]
            for jj in range(njb // 2):
                jb0 = 2 * jj
                first, last = jj == 0, jj == njb // 2 - 1
                p_tiles = []
                for pr in range(2):
                    s_h = [pp.tile([128, 1024], F32, name=f"s{e}", tag="s", bufs=2)
                           for e in range(2)]
                    # S^T: row-tiled head pair, two j-blocks side by side
                    for half, jb in enumerate((jb0, jb0 + 1)):
                        jsl = slice(128 * jb, 128 * jb + 128)
                        fsl = slice(512 * half, 512 * half + 512)
                        nc.tensor.matmul(s_h[0][:, fsl], lhsT=kT[pr][0:64, jsl],
                                         rhs=qT[pr][0:64, isl],
                                         start=True, stop=True, tile_position=(0, 0))
                        nc.tensor.matmul(s_h[1][:, fsl], lhsT=kT[pr][64:128, jsl],
                                         rhs=qT[pr][64:128, isl],
                                         start=True, stop=True, tile_position=(64, 0))
                    for e in range(2):
                        h = 2 * pr + e
                        p_t = pb.tile([128, 1024], BF16, name="p_t", tag="p", bufs=6)
                        nc.scalar.activation(p_t, s_h[e], AF.Exp, scale=SCALE)
                        for half, jb in enumerate((jb0, jb0 + 1)):
                            off = _off(it, jb)
                            if 0 <= off < 512:
                                fsl = slice(512 * half, 512 * half + 512)
                                nc.vector.tensor_tensor(
                                    p_t[:, fsl], p_t[:, fsl],
                                    masks[:, off // 128, :], MULT)
                        p_tiles.append(p_t)
                        for half, jb in enumerate((jb0, jb0 + 1)):
                            fsl = slice(512 * half, 512 * half + 512)
                            nc.tensor.matmul(
                                pv[pr][64 * e:64 * e + 64, :],
                                lhsT=vp[:, jb, 64 * h:64 * h + 64],
                                rhs=p_t[:, fsl],
                                start=(first and half == 0), stop=(last and half == 1),
                                tile_position=(0, 64 * e), skip_group_check=True)
                # denominators: 4 heads col-tiled concurrently, per j-half
                for half in range(2):
                    if "nol" in ABLATE and not (first or last):
                        continue
                    fsl = slice(512 * half, 512 * half + 512)
                    for h in range(4):
                        nc.tensor.matmul(
                            lp[32 * h:32 * h + 1, :], lhsT=ones,
                            rhs=p_tiles[h][:, fsl],
                            start=(first and half == 0), stop=(last and half == 1),
                            tile_position=(0, 32 * h), skip_group_check=True)
            # 1/l, broadcast via DRAM bounce, then scale pv -> ao
            linv = ob.tile([128, 512], F32)
            for h in range(4):
                nc.vector.reciprocal(out=linv[32 * h:32 * h + 1, :],
                                     in_=lp[32 * h:32 * h + 1, :])
            ltmp = dp.tile([4, 512], F32)
            nc.sync.dma_start(
                out=ltmp,
                in_=linv.rearrange("(a b) f -> a b f", b=32)[:, 0, :])
            for pr in range(2):
                bc = ob.tile([128, 512], F32)
                for e in range(2):
                    h = 2 * pr + e
                    nc.sync.dma_start(out=bc[64 * e:64 * e + 64, :],
                                      in_=ltmp[h:h + 1, :].partition_broadcast(64))
                nc.vector.tensor_tensor(ao[pr][:, isl], pv[pr], bc, MULT)

        # ---- output projection (partial product over this core's heads) ----
        for ib in range(N // 128):
            for nb in range(2):
                ps = pp.tile([128, 512], F32, name="ps_o", tag="A", bufs=2)
                for pr in range(2):
                    nc.tensor.matmul(
                        ps, lhsT=ao[pr][:, 128 * ib:128 * ib + 128],
                        rhs=wo[:, pr, 512 * nb:512 * nb + 512],
                        start=(pr == 0), stop=(pr == 1))
                ot = ob.tile([128, 512], F32)
                nc.vector.tensor_copy(out=ot, in_=ps)
                nc.sync.dma_start(
                    out=out_d[128 * ib:128 * ib + 128, 512 * nb:512 * nb + 512],
                    in_=ot)
        if rep_cm is not None:
            rep_cm.__exit__(None, None, None)
    nc.compile()
    return nc


_NC = None


def _get_nc():
    global _NC
    if _NC is None:
        _NC = build_program()
    return _NC


def _make_masks():
    m = np.zeros((4, 128, 512), dtype=ml_dtypes.bfloat16)
    fi = np.arange(512)[None, :]
    fj = np.arange(128)[:, None]
    for o in range(4):
        m[o] = (fi >= fj + 128 * o).astype(ml_dtypes.bfloat16)
    return m


def make_in_maps(x, mem, expire_mask, Wq, Wkv, Wo):
    bf = ml_dtypes.bfloat16
    masks = _make_masks()
    ctxT = []
    for b in range(B):
        c = np.concatenate([mem[b], x[b]], axis=0)          # [J, DIM]
        ctxT.append(np.ascontiguousarray(c.T).astype(bf))   # [DIM, J]

    in_maps = []
    for core in range(NCORES):
        b, hg = core // 4, core % 4
        cs = slice(DHC * hg, DHC * hg + DHC)
        in_maps.append({
            "ctxT": ctxT[b],
            "wq": np.ascontiguousarray(Wq[:, cs]).astype(bf),
            "wk": np.ascontiguousarray(Wkv[:, cs]).astype(bf),
            "wv": np.ascontiguousarray(Wkv[:, DIM + cs.start:DIM + cs.stop]).astype(bf),
            "wo": np.ascontiguousarray(Wo[cs, :]).astype(bf),
            "expire": np.ascontiguousarray(expire_mask[b, 0, 0].reshape(NJB, 128)),
            "masks": masks,
        })
    return in_maps


def kernel(x, mem, expire_mask, Wq, Wkv, Wo, bo):
    x = np.asarray(x, dtype=np.float32)
    mem = np.asarray(mem, dtype=np.float32)
    expire_mask = np.asarray(expire_mask, dtype=np.float32)
    Wq = np.asarray(Wq, dtype=np.float32)
    Wkv = np.asarray(Wkv, dtype=np.float32)
    Wo = np.asarray(Wo, dtype=np.float32)
    bo = np.asarray(bo, dtype=np.float32)

    in_maps = make_in_maps(x, mem, expire_mask, Wq, Wkv, Wo)
    nc = _get_nc()
    res = run_bass_kernel_spmd(nc, in_maps, core_ids=list(range(NCORES)))

    out = np.zeros((B, N, DIM), dtype=np.float32)
    for core in range(NCORES):
        out[core // 4] += res.results[core]["out"]
    out += bo[None, None, :]
    return out

